# revision 28
# baseline (speedup 1.0000x reference)
"""Bidirectional Mamba2 layer on 8 NeuronCores.

Sharding: 8 cores = 4 batch elements x 2 directions (fw/bw). Each core runs
one full Mamba2 layer pass on one sequence; the host flips the bw sequences,
adds fw+bw results, and applies the padding mask.

Per-core kernel (L=2048, chunked SSD scan with T=128), v2:
  1. in_proj as channel-major matmuls (bf16): silu(z), pre-conv xBC, dt.
  2. depthwise conv width-4 on Vector (overlapped under in_proj PE work),
     D-residual yD = D*x precomputed globally.
  3. DRAM roundtrip with DMA-transpose for time-major x, B per chunk.
  4. chunked scan with batched broadcast ops: per chunk one utmp build
     (V+G split), one seg-sum matmul group, batched exp, one masked-G
     multiply for M, batched state matmul, C consumed channel-major with
     inter-chunk decay applied post-PSUM via a DMA-broadcast a_bc tile.
  5. gating + RMSNorm interleaved per chunk (norm_w folded into out_proj
     weights on the host), out_proj at the tail.
"""

import numpy as np

D_MODEL = 512
D_STATE = 128
NH = 16
HD = 64
D_INNER = 1024
D_XBC = 1280
D_IN = 2320
L = 2048
T = 128
NCH = L // T
B_SZ = 4
EPS = 1e-5

_CACHE = {}


def _patch_drain(tile, mybir, ScopedClock):
    # workaround: this walrus build rejects >2 sem waits per instruction;
    # spread the TileContext exit-drain waits across nop instructions.
    def _drain_and_barrier(self, tick_clock, wait_clock):
        nc_ = self.nc
        probe = nc_.sync.nop()
        wait_clock.add_sem_waits(
            probe.ins, ScopedClock({None: tick_clock.global_clock})
        )
        waits = list(probe.ins.sync_info.on_wait or [])
        if probe.ins.sync_info is not None:
            probe.ins.sync_info.on_wait = waits[:1]
            rest = waits[1:]
        else:
            rest = []
        for w in rest:
            n = nc_.sync.nop()
            if n.ins.sync_info is None:
                n.ins.sync_info = mybir.SyncInfo(on_wait=[w], on_update=[])
            else:
                n.ins.sync_info.on_wait = [w]
        nc_.sync.drain()
        nc_.all_engine_barrier()
        assert self.sems is not None
        popped = nc_._tile_sem_poison_stack.pop()
        assert popped is self._sem_poison
        nc_.clear_and_free_semaphores(list(self.sems.allocated().values()))
        nc_.all_engine_barrier()

    tile.TileContext._drain_and_barrier = _drain_and_barrier


def _build_program():
    import concourse.bass as bass
    import concourse.mybir as mybir
    import concourse.tile as tile
    from concourse.vector_clock import ScopedClock

    _patch_drain(tile, mybir, ScopedClock)

    f32 = mybir.dt.float32
    bf16 = mybir.dt.bfloat16
    AF = mybir.ActivationFunctionType
    OP = mybir.AluOpType

    nc = bass.Bass("TRN2", target_bir_lowering=False, debug=False)

    # ---------------- DRAM I/O ----------------
    xT_d = nc.dram_tensor("xT", [D_MODEL, L], bf16, kind="ExternalInput")
    w_in_d = nc.dram_tensor("w_in", [D_MODEL, D_IN], bf16, kind="ExternalInput")
    w_out_d = nc.dram_tensor("w_out", [D_INNER, D_MODEL], bf16, kind="ExternalInput")
    cwdiag_d = nc.dram_tensor("cwdiag", [128, 32, 128], bf16, kind="ExternalInput")
    idnb_d = nc.dram_tensor("idnb", [128, 128], bf16, kind="ExternalInput")
    convw_d = nc.dram_tensor("convw", [128, 10, 4], f32, kind="ExternalInput")
    convb_d = nc.dram_tensor("convb", [128, 10], f32, kind="ExternalInput")
    dtb_d = nc.dram_tensor("dtb", [16, 1], f32, kind="ExternalInput")
    nae_d = nc.dram_tensor("nae", [16, 1], f32, kind="ExternalInput")  # -exp(A_log)
    dcol_d = nc.dram_tensor("dcol", [128, 8], f32, kind="ExternalInput")  # D per pair-tile
    alow_d = nc.dram_tensor("alow", [128, 128], bf16, kind="ExternalInput")
    uinc_d = nc.dram_tensor("uinc", [128, 128], bf16, kind="ExternalInput")
    idnf_d = nc.dram_tensor("idnf", [128, 128], f32, kind="ExternalInput")
    ones_d = nc.dram_tensor("ones", [128, 1], bf16, kind="ExternalInput")
    onesrf_d = nc.dram_tensor("onesrf", [1, 128], f32, kind="ExternalInput")
    onesrb_d = nc.dram_tensor("onesrb", [1, 128], bf16, kind="ExternalInput")
    yT_d = nc.dram_tensor("yT", [D_MODEL, L], f32, kind="ExternalOutput")

    with tile.TileContext(nc) as tc:
        with (
            tc.tile_pool(name="const", bufs=1) as cpool,
            tc.tile_pool(name="dram", bufs=1, space="DRAM") as dpool,
            tc.tile_pool(name="mid", bufs=1) as mid,
        ):
            # ---------------- constants ----------------
            ALOW = cpool.tile([128, 128], bf16, tag="alow")
            nc.sync.dma_start(ALOW[:], alow_d.ap())
            UINC = cpool.tile([128, 128], bf16, tag="uinc")
            nc.sync.dma_start(UINC[:], uinc_d.ap())
            IDNB = cpool.tile([128, 128], bf16, tag="idnb")
            nc.sync.dma_start(IDNB[:], idnb_d.ap())
            CWDIAG = cpool.tile([128, 32, 128], bf16, tag="cwdiag")
            nc.sync.dma_start(CWDIAG[:], cwdiag_d.ap())
            IDNF = cpool.tile([128, 128], f32, tag="idnf")
            nc.sync.dma_start(IDNF[:], idnf_d.ap())
            ONEC = cpool.tile([128, 1], bf16, tag="ones")
            nc.sync.dma_start(ONEC[:], ones_d.ap())
            ONESRF = cpool.tile([1, 128], f32, tag="onesrf")
            nc.sync.dma_start(ONESRF[:], onesrf_d.ap())
            ONESRB = cpool.tile([1, 128], bf16, tag="onesrb")
            nc.sync.dma_start(ONESRB[:], onesrb_d.ap())
            CONVW = cpool.tile([128, 10, 4], f32, tag="convw")
            nc.sync.dma_start(CONVW[:], convw_d.ap())
            CONVB = cpool.tile([128, 10], f32, tag="convb")
            nc.sync.dma_start(CONVB[:], convb_d.ap())
            DTB = cpool.tile([16, 1], f32, tag="dtb")
            nc.sync.dma_start(DTB[:], dtb_d.ap())
            NAE = cpool.tile([16, 1], f32, tag="nae")
            nc.sync.dma_start(NAE[:], nae_d.ap())
            DCOL = cpool.tile([128, 8], f32, tag="dcol")
            nc.sync.dma_start(DCOL[:], dcol_d.ap())
            EPSC = cpool.tile([128, 1], f32, tag="epsc")
            nc.vector.memset(EPSC[:], EPS)

            # ---------------- persistent mid tensors ----------------
            dtldT = mid.tile([128, NCH, 96], f32, tag="dtldT")  # time-major per chunk
            atot = mid.tile([16, 16], f32, tag="atot")          # [head, chunk]
            atotT = mid.tile([16, 16], f32, tag="atotT")
            atotF = mid.tile([1, 256], f32, tag="atotF")
            s_sb = [mid.tile([128, NH, HD], bf16, tag=f"s_sb{i}", name=f"s_sb{i}")
                    for i in range(2)]
            atb_all = mid.tile([128, NCH, 16], f32, tag="atb_all")    # atot bcast
            sz = mid.tile([128, 8, L], bf16, tag="sz")                # silu(z)
            rstd_cols = mid.tile([128, 16], f32, tag="rstd_cols")

            rt_dram = dpool.tile([2 * D_INNER + D_STATE, L], bf16)  # xdt,xdw,B
            aT_dram = dpool.tile([NCH, 16, 128], bf16)          # prefix decay aT
            suf_dram = dpool.tile([NCH, 16, 128], bf16)         # suffix decay
            dt_dram = dpool.tile([16, L], bf16)                 # dt rows
            yD_dram = dpool.tile([D_INNER, L], bf16)            # D*x residual

            xbc_B = mid.tile([128, L], bf16, tag="xbc_B")
            xbc_C = mid.tile([128, L], bf16, tag="xbc_C")

            # ============ PHASE 1: in_proj + conv + decay prep ============
            with tc.tile_pool(name="p_ph1", bufs=1) as p1:
                dtld = p1.tile([96, L], f32, tag="dtld")  # dt 0:16, logdA 64:80
                xbc_pre = [p1.tile([128, L + 3], bf16, tag=f"xbc_pre{t}",
                                   name=f"xbc_pre{t}") for t in range(10)]
                with (
                    tc.tile_pool(name="pA", bufs=1) as pA,
                    tc.tile_pool(name="ps1", bufs=4, space="PSUM") as ps1,
                    tc.tile_pool(name="pss", bufs=2, space="PSUM") as pss,
                    tc.tile_pool(name="pC", bufs=2) as pC,
                    tc.tile_pool(name="pC1", bufs=1) as pC1,
                    tc.tile_pool(name="pD", bufs=2) as pDp,
                    tc.tile_pool(name="pDT", bufs=1) as pDT,
                ):
                    xTr = xT_d.ap().rearrange("(ko p) t -> p ko t", p=128)
                    wir = w_in_d.ap().rearrange("(ko p) m -> p ko m", p=128)
                    xTs = pA.tile([128, 4, L], bf16, tag="xTs")
                    wis = pA.tile([128, 4, D_IN], bf16, tag="wis")
                    for k in range(4):
                        nc.sync.dma_start(xTs[:, k, :], xTr[:, k, :])
                        nc.sync.dma_start(wis[:, k, :], wir[:, k, :])

                    for t in range(10):
                        nc.vector.memset(xbc_pre[t][:, 0:3], 0.0)

                    def do_conv_x(t):
                        # x tiles: conv as 4 accumulating diag-matmuls per tb
                        outs = []
                        for tb in range(4):
                            csp = ps1.tile([128, 512], f32, tag="ps_inproj",
                                           name=f"cv{t}_{tb}")
                            for k in range(4):
                                nc.tensor.matmul(
                                    csp[:], CWDIAG[:, t * 4 + k, :],
                                    xbc_pre[t][:, k + tb * 512: k + (tb + 1) * 512],
                                    start=(k == 0), stop=(k == 3))
                            outs.append(csp)
                        dst = pC.tile([128, L], bf16, tag="conv_x")
                        for tb in range(4):
                            nc.scalar.activation(
                                dst[:, tb * 512:(tb + 1) * 512], outs[tb][:],
                                AF.Silu, bias=CONVB[:, t:t + 1])
                        # xdt = x*dt ; xdw = xdt*suffix  (channel-major, 2x TT)
                        dtc = pC1.tile([128, L], bf16, tag="dt_cm")
                        for hh in range(2):
                            nc.sync.dma_start(
                                dtc[hh * 64:(hh + 1) * 64, :],
                                dt_dram[2 * t + hh][None, :].to_broadcast([64, L]))
                        sfc = pC1.tile([128, NCH, 128], bf16, tag="suf_cm")
                        for hh in range(2):
                            nc.sync.dma_start(
                                sfc[hh * 64:(hh + 1) * 64, :, :],
                                suf_dram[:, 2 * t + hh, :][None, :, :]
                                .to_broadcast([64, NCH, 128]))
                        xdt = pC.tile([128, L], bf16, tag="xdt_cm")
                        nc.vector.tensor_tensor(xdt[:], dst[:], dtc[:], op=OP.mult)
                        xdw = pC1.tile([128, L], bf16, tag="xdw_cm")
                        nc.vector.tensor_tensor(
                            xdw[:], xdt[:],
                            sfc[:].rearrange("p c i -> p (c i)"), op=OP.mult)
                        nc.sync.dma_start(rt_dram[t * 128:(t + 1) * 128, :], xdt[:])
                        nc.sync.dma_start(
                            rt_dram[1024 + t * 128:1024 + (t + 1) * 128, :], xdw[:])
                        # D residual via DRAM
                        yDt = pC1.tile([128, L], bf16, tag="conv_yd")
                        nc.vector.tensor_scalar_mul(
                            yDt[:], dst[:], DCOL[:, t:t + 1])
                        nc.sync.dma_start(yD_dram[t * 128:(t + 1) * 128, :], yDt[:])

                    def do_conv_bc(t):
                        # B/C tiles: vector chain
                        acc = pC1.tile([128, L], bf16, tag="conv_acc")
                        nc.vector.tensor_scalar_mul(
                            acc[:], xbc_pre[t][:, 0:L], CONVW[:, t, 0:1])
                        for k in range(1, 4):
                            nc.vector.scalar_tensor_tensor(
                                acc[:], xbc_pre[t][:, k:k + L],
                                CONVW[:, t, k:k + 1], acc[:],
                                op0=OP.mult, op1=OP.add,
                            )
                        dst = xbc_B if t == 8 else xbc_C
                        nc.scalar.activation(dst[:], acc[:], AF.Silu,
                                             bias=CONVB[:, t:t + 1])
                        if t == 8:
                            nc.sync.dma_start(rt_dram[2048:2176, :], dst[:])

                    for m in [18] + list(range(8, 18)) + list(range(0, 8)):
                        mp = 128 if m < 18 else 16
                        if m < 18:
                            # stationary (m,k) reused across the 4 tb blocks
                            pstb = [ps1.tile([128, 512], f32, tag="ps_inproj",
                                             name=f"ip{m}_{tb}") for tb in range(4)]
                            for k in range(4):
                                for tb in range(4):
                                    nc.tensor.matmul(
                                        pstb[tb][:],
                                        wis[:, k, m * 128: m * 128 + mp],
                                        xTs[:, k, tb * 512:(tb + 1) * 512],
                                        start=(k == 0), stop=(k == 3))
                            for tb in range(4):
                                tsl = slice(tb * 512, (tb + 1) * 512)
                                if m < 8:
                                    nc.scalar.activation(sz[:, m, tsl], pstb[tb][:],
                                                         AF.Silu)
                                else:
                                    t = m - 8
                                    nc.scalar.copy(
                                        xbc_pre[t][:, 3 + tb * 512: 3 + (tb + 1) * 512],
                                        pstb[tb][:])
                        else:
                            for tb in range(4):
                                tsl = slice(tb * 512, (tb + 1) * 512)
                                ps = ps1.tile([128, 512], f32, tag="ps_inproj")
                                for k in range(4):
                                    nc.tensor.matmul(
                                        ps[:mp, :],
                                        wis[:, k, m * 128: m * 128 + mp],
                                        xTs[:, k, tsl],
                                        start=(k == 0), stop=(k == 3))
                                nc.scalar.copy(dtld[32:48, tsl], ps[:16, :])
                        if m == 18:
                            # dt = softplus(pre) = ln(1 + exp(pre + dtb))
                            nc.scalar.activation(dtld[32:48, :], dtld[32:48, :], AF.Exp,
                                                 bias=DTB[:, 0:1])
                            nc.scalar.activation(dtld[0:16, :], dtld[32:48, :], AF.Ln,
                                                 bias=1.0)
                            # logdA = -exp(A_log) * dt   (f32)
                            nc.vector.tensor_scalar_mul(
                                dtld[64:80, :], dtld[0:16, :], NAE[:, 0:1])
                            # dt rows to DRAM for channel-major broadcast
                            dtbf = pDT.tile([16, L], bf16, tag="dtbf")
                            nc.vector.tensor_copy(dtbf[:], dtld[0:16, :])
                            nc.sync.dma_start(dt_dram[:], dtbf[:])

                            # Atot per chunk = exp(chunk-sums of logdA)
                            red = pss.tile([128, 32], f32, tag="small", name="red")
                            nc.vector.tensor_reduce(
                                red[0:16, 0:16],
                                dtld[64:80, :].rearrange("p (c t) -> p c t", c=NCH),
                                op=OP.add, axis=mybir.AxisListType.X,
                            )
                            nc.scalar.activation(atot[:], red[0:16, 0:16], AF.Exp)
                            atT_ps = pss.tile([128, 32], f32, tag="small", name="atT_ps")
                            nc.tensor.transpose(
                                atT_ps[0:16, 0:16], atot[:], IDNF[0:16, 0:16])
                            nc.vector.tensor_copy(atotT[:], atT_ps[0:16, 0:16])
                            nc.sync.dma_start(
                                atotF[:].rearrange("p (c h) -> p c h", c=16), atotT[:])

                            # time-major dt/logdA per chunk via PE transpose
                            for c in range(NCH):
                                trp = pss.tile([128, 128], f32, tag="small2", name="trp")
                                nc.tensor.transpose(
                                    trp[:, 0:96], dtld[:, c * T:(c + 1) * T],
                                    IDNF[0:96, 0:96])
                                nc.vector.tensor_copy(dtldT[:, c, :], trp[:, 0:96])

                            # ---- decay prep (all chunks up front) ----
                            for c in range(NCH):
                                ld_bf = pDp.tile([128, 16], bf16, tag="ld_bf")
                                nc.vector.tensor_copy(ld_bf[:], dtldT[:, c, 64:80])
                                # suffix decay sufT[h,t]=exp(sum_{k>t} logdA)
                                sfT_ps = pss.tile([128, 128], f32, tag="small2",
                                                  name="sfT_ps")
                                nc.tensor.matmul(sfT_ps[0:16, :], ld_bf[:], ALOW[:],
                                                 start=True, stop=True)
                                sfT_sb = pDp.tile([16, 128], bf16, tag="sfT_sb")
                                nc.scalar.activation(sfT_sb[:], sfT_ps[0:16, :], AF.Exp)
                                nc.sync.dma_start(suf_dram[c], sfT_sb[:])
                                # prefix decay aT[h, t] = exp(inclusive cumsum)
                                csT_ps = pss.tile([128, 128], f32, tag="small2",
                                                  name="csT_ps")
                                nc.tensor.matmul(csT_ps[0:16, :], ld_bf[:], UINC[:],
                                                 start=True, stop=True)
                                aT_sb = pDp.tile([16, 128], bf16, tag="aT_sb")
                                nc.scalar.activation(aT_sb[:], csT_ps[0:16, :], AF.Exp)
                                nc.sync.dma_start(aT_dram[c], aT_sb[:])
                                if c > 0:
                                    at_ps = pss.tile([128, 32], f32, tag="small",
                                                     name="at_ps")
                                    nc.tensor.matmul(
                                        at_ps[:, 0:16], ONESRF[:],
                                        atotF[0:1, c * 16:(c + 1) * 16],
                                        start=True, stop=True)
                                    nc.vector.tensor_copy(atb_all[:, c, :],
                                                          at_ps[:, 0:16])

                    # conv interleaved (tiles ready in production order)
                    for t in range(8):
                        do_conv_x(t)
                    do_conv_bc(8)
                    do_conv_bc(9)

            # ============ PHASE 2: chunked scan + gating ============
            with tc.tile_pool(name="p_gn", bufs=1) as pgn:
              gn = pgn.tile([128, 8, L], bf16, tag="gn")
              with (
                tc.tile_pool(name="pS", bufs=2) as pS,
                tc.tile_pool(name="pXT", bufs=3) as pXT,
                tc.tile_pool(name="pAB", bufs=2) as pAB,
                tc.tile_pool(name="psE", bufs=2, space="PSUM") as psE,
                tc.tile_pool(name="psY1", bufs=1, space="PSUM") as psY1,
                tc.tile_pool(name="psY2", bufs=1, space="PSUM") as psY2,
                tc.tile_pool(name="psS", bufs=1, space="PSUM") as psS,
              ):
                gn_prev = [None]

                def emit_gn(cp, g_p, rb_p):
                    # gn = g * rstd (norm_w folded into W_out on host)
                    nc.vector.tensor_tensor(
                        gn[:, :, cp * T:(cp + 1) * T], g_p[:],
                        rb_p[:, None, :].to_broadcast([128, 8, T]), op=OP.mult)

                for c in range(NCH):
                    csl = slice(c * T, (c + 1) * T)

                    # time-major xdt, xdw, B for this chunk
                    xbt = pXT.tile([128, 2 * D_INNER + D_STATE], bf16, tag="xbt")
                    nc.sync.dma_start_transpose(xbt[:, 0:1024], rt_dram[0:1024, csl])
                    nc.sync.dma_start_transpose(xbt[:, 1024:2048],
                                                rt_dram[1024:2048, csl])
                    nc.sync.dma_start_transpose(xbt[:, 2048:2176],
                                                rt_dram[2048:2176, csl])
                    yD_c = pXT.tile([128, 8, T], bf16, tag="yD_c")
                    nc.sync.dma_start(
                        yD_c[:],
                        yD_dram[:].rearrange("(t8 p) l -> p t8 l", p=128)[:, :, csl])

                    # a_bc[p, t8, i] = aT[2*t8 + p//64, i]
                    if c > 0:
                        a_bc = pAB.tile([128, 8, 128], bf16, tag="a_bc")
                        for hh in range(2):
                            nc.sync.dma_start(
                                a_bc[hh * 64:(hh + 1) * 64, :, :],
                                aT_dram[c, hh::2, :][None, :, :]
                                .to_broadcast([64, 8, 128]))

                    # deferred gn of previous chunk
                    if gn_prev[0] is not None:
                        emit_gn(*gn_prev[0])
                        gn_prev[0] = None

                    # utmp[k, h, i] = uinc[k, i] * logdA[k, h]  (V half, G half)
                    utmp = pS.tile([128, NH, 128], bf16, tag="utmp")
                    nc.vector.tensor_tensor(
                        utmp[:, 0:8, :],
                        UINC[:, None, :].to_broadcast([128, 8, 128]),
                        dtldT[:, c, 64:72, None].to_broadcast([128, 8, 128]),
                        op=OP.mult)
                    nc.gpsimd.tensor_tensor(
                        utmp[:, 8:16, :],
                        UINC[:, None, :].to_broadcast([128, 8, 128]),
                        dtldT[:, c, 72:80, None].to_broadcast([128, 8, 128]),
                        op=OP.mult)

                    # Gt = B^T C (shared across heads), masked to lower-incl
                    gt_ps = psE.tile([128, 512], f32, tag="psE", name="gt_ps")
                    nc.tensor.matmul(gt_ps[:, 0:128], xbc_B[:, csl], xbc_C[:, csl],
                                     start=True, stop=True)
                    gtm = pS.tile([128, 128], bf16, tag="gtm")
                    nc.vector.tensor_tensor(gtm[:], gt_ps[:, 0:128], UINC[:],
                                            op=OP.mult)

                    # segment sums + exp -> E, then M = gtm * E
                    e_all = pS.tile([128, NH, 128], bf16, tag="e_all")
                    for q in range(4):
                        e_ps = psE.tile([128, 512], f32, tag="psE", name="e_ps")
                        nc.tensor.matmul(e_ps[:], ALOW[:],
                                         utmp[:, 4 * q:4 * (q + 1), :],
                                         start=True, stop=True)
                        nc.scalar.activation(e_all[:, 4 * q:4 * (q + 1), :],
                                             e_ps[:], AF.Exp)
                    m_all = pS.tile([128, NH, 128], bf16, tag="m_all")
                    nc.vector.tensor_tensor(
                        m_all[:], e_all[:],
                        gtm[:, None, :].to_broadcast([128, NH, 128]), op=OP.mult)

                    # Y accumulation
                    y1_ps = psY1.tile([128, 8, T], f32, tag="y1_ps")
                    for h in range(NH):
                        ph, fh = (h % 2) * 64, h // 2
                        nc.tensor.matmul(y1_ps[ph:ph + 64, fh, :],
                                         xbt[:, h * HD:(h + 1) * HD],
                                         m_all[:, h, :],
                                         start=True, stop=True)
                    if c > 0:
                        y2_ps = psY2.tile([128, 8, T], f32, tag="y2_ps")
                        for h in range(NH):
                            ph, fh = (h % 2) * 64, h // 2
                            nc.tensor.matmul(y2_ps[ph:ph + 64, fh, :],
                                             s_sb[(c + 1) % 2][:, h, :],
                                             xbc_C[:, csl],
                                             start=True, stop=True)

                    # state: S = B^T @ xdw, then + S_prev*atot
                    s_ps = psS.tile([128, NH, HD], f32, tag="s_ps", name="s_ps")
                    for half in range(2):
                        hsl = slice(8 * half, 8 * (half + 1))
                        nc.tensor.matmul(
                            s_ps[:, hsl, :].rearrange("p h d -> p (h d)"),
                            xbt[:, 2048:2176],
                            xbt[:, 1024 + 512 * half:1024 + 512 * (half + 1)],
                            start=True, stop=True)
                    if c == 0:
                        nc.vector.tensor_copy(s_sb[0][:], s_ps[:])
                    else:
                        s_scaled = pS.tile([128, NH, HD], bf16, tag="s_scaled")
                        nc.gpsimd.tensor_tensor(
                            s_scaled[:], s_sb[(c + 1) % 2][:],
                            atb_all[:, c, :, None].to_broadcast([128, NH, HD]),
                            op=OP.mult)
                        nc.vector.tensor_tensor(
                            s_sb[c % 2][:], s_scaled[:], s_ps[:], op=OP.add)

                    # evac: y = y1 + a_bc*y2 + yD, then gate g = y*sz
                    g_sb = pS.tile([128, 8, T], bf16, tag="g_sb")
                    if c > 0:
                        y2s = pS.tile([128, 8, T], bf16, tag="y2s")
                        nc.vector.tensor_tensor(y2s[:], y2_ps[:], a_bc[:],
                                                op=OP.mult)
                        tmp = pS.tile([128, 8, T], bf16, tag="tmp")
                        nc.vector.tensor_tensor(tmp[:], y1_ps[:], y2s[:], op=OP.add)
                        yf = pS.tile([128, 8, T], bf16, tag="yf")
                        nc.vector.tensor_tensor(yf[:], tmp[:], yD_c[:], op=OP.add)
                    else:
                        yf = pS.tile([128, 8, T], bf16, tag="yf")
                        nc.vector.tensor_tensor(yf[:], y1_ps[:], yD_c[:], op=OP.add)
                    if c % 2 == 0:
                        nc.gpsimd.tensor_tensor(g_sb[:], yf[:], sz[:, :, csl],
                                                op=OP.mult)
                    else:
                        nc.vector.tensor_tensor(g_sb[:], yf[:], sz[:, :, csl],
                                                op=OP.mult)

                    # RMS stats: rstd per time (PE partition-reduce)
                    g2 = pS.tile([128, 8, T], bf16, tag="g2")
                    nc.scalar.square(g2[:], g_sb[:])
                    ss_ps = psS.tile([128, NH * HD], f32, tag="s_ps", name="ss_ps")
                    for t8 in range(8):
                        nc.tensor.matmul(ss_ps[:, 0:1], g2[:, t8, :], ONEC[:],
                                         start=(t8 == 0), stop=(t8 == 7))
                    lnv = pS.tile([128, 1], f32, tag="lnv")
                    nc.scalar.activation(lnv[:], ss_ps[:, 0:1], AF.Ln,
                                         bias=EPSC[:, 0:1], scale=1.0 / D_INNER)
                    nc.scalar.activation(rstd_cols[:, c:c + 1], lnv[:],
                                         AF.Exp, scale=-0.5)
                    # broadcast rstd over partitions: transpose + 1-row matmul
                    rs_ps = psS.tile([128, NH * HD], f32, tag="s_ps", name="rs_ps")
                    nc.tensor.transpose(rs_ps[0:1, 0:128],
                                        rstd_cols[:, c:c + 1], IDNF[:])
                    rsT = pS.tile([1, 128], bf16, tag="rsT")
                    nc.vector.tensor_copy(rsT[:], rs_ps[0:1, 0:128])
                    rb_ps = psS.tile([128, NH * HD], f32, tag="s_ps", name="rb_ps")
                    nc.tensor.matmul(rb_ps[:, 0:128], ONESRB[:], rsT[:],
                                     start=True, stop=True)
                    rb_sb = pS.tile([128, 128], bf16, tag="rb_sb")
                    nc.vector.tensor_copy(rb_sb[:], rb_ps[:, 0:128])
                    gn_prev[0] = (c, g_sb, rb_sb)

                # final chunk's gn
                if gn_prev[0] is not None:
                    emit_gn(*gn_prev[0])
                    gn_prev[0] = None

              # ============ PHASE 3: out_proj ============
              with (
                tc.tile_pool(name="pO", bufs=1) as pO,
                tc.tile_pool(name="psO", bufs=4, space="PSUM") as psO,
              ):
                wo = pO.tile([128, 8, D_MODEL], bf16, tag="wo")
                nc.sync.dma_start(
                    wo[:], w_out_d.ap().rearrange("(k p) m -> p k m", p=128))
                yT_sb = pO.tile([128, 4, L], f32, tag="yT_sb")
                for m in range(4):
                    pstb = [psO.tile([128, 512], f32, tag="ps_out",
                                     name=f"op{m}_{tb}") for tb in range(4)]
                    for k in range(8):
                        for tb in range(4):
                            nc.tensor.matmul(
                                pstb[tb][:], wo[:, k, m * 128:(m + 1) * 128],
                                gn[:, k, tb * 512:(tb + 1) * 512],
                                start=(k == 0), stop=(k == 7))
                    for tb in range(4):
                        nc.scalar.copy(yT_sb[:, m, tb * 512:(tb + 1) * 512],
                                       pstb[tb][:])
                nc.sync.dma_start(
                    yT_d.ap().rearrange("(mo p) t -> p mo t", p=128), yT_sb[:])

    _fix_waits(nc, mybir)

    return nc


def _fix_waits(nc, mybir):
    """This walrus build supports one sem-wait slot per instruction; hoist
    excess waits onto preceding NoOps on the same engine."""
    nwn = [0]
    for bb in nc.main_func.blocks:
        newl = []
        changed = False
        for inst in bb.instructions:
            si = inst.sync_info
            waits = list(si.on_wait) if (si and si.on_wait) else []
            if len(waits) > 1:
                imm = [w for w in waits if w.wait_reg is None]
                reg = [w for w in waits if w.wait_reg is not None]
                keep = (reg + imm)[:1]
                spill = [w for w in waits if w not in keep]
                assert not any(w.wait_reg is not None for w in spill), inst.name
                for w in spill:
                    nwn[0] += 1
                    nop = mybir.InstNoOp(name=f"I-wsplit-{nwn[0]}", ins=[], outs=[])
                    nop.engine = inst.engine
                    nop.sync_info = mybir.SyncInfo(on_wait=[w], on_update=[])
                    nc.register_instruction(nop)
                    newl.append(nop)
                si.on_wait = keep
                changed = True
            newl.append(inst)
        if changed:
            bb.instructions = newl
    return nc


def _get_program():
    if "nc" not in _CACHE:
        _CACHE["nc"] = _build_program()
    return _CACHE["nc"]


def _host_consts():
    if "consts" in _CACHE:
        return _CACHE["consts"]
    import ml_dtypes
    k = np.arange(128)
    alow = (k[:, None] > k[None, :]).astype(np.float32)      # [k > j]
    uinc = (k[:, None] <= k[None, :]).astype(np.float32)     # [k <= i]
    idn = np.eye(128, dtype=np.float32)
    consts = dict(
        alow=alow.astype(ml_dtypes.bfloat16),
        uinc=uinc.astype(ml_dtypes.bfloat16),
        idnb=idn.astype(ml_dtypes.bfloat16),
        idnf=idn,
        ones=np.ones((128, 1), ml_dtypes.bfloat16),
        onesrf=np.ones((1, 128), np.float32),
        onesrb=np.ones((1, 128), ml_dtypes.bfloat16),
    )
    _CACHE["consts"] = consts
    return consts


def _core_inputs(x_seq, p):
    """x_seq: (L, D_MODEL) f32 (already flipped for bw); p: dict of params."""
    import ml_dtypes
    consts = _host_consts()
    dcol = p["D"].astype(np.float32).repeat(HD).reshape(8, 128).T.copy()
    convw = np.ascontiguousarray(
        p["conv_w"].astype(np.float32).reshape(4, 10, 128).transpose(2, 1, 0)
    )
    convb = np.ascontiguousarray(p["conv_b"].astype(np.float32).reshape(10, 128).T)
    # diag conv weights for the 8 x-tiles: cwdiag[p, t*4+k, m] = d_pm * w[k, t*128+p]
    cwdiag = np.zeros((128, 32, 128), np.float32)
    cw = p["conv_w"].astype(np.float32)  # [4, 1280]
    for t in range(8):
        for k in range(4):
            np.fill_diagonal(cwdiag[:, t * 4 + k, :], cw[k, t * 128:(t + 1) * 128])
    w_out = (p["norm_w"].astype(np.float32)[:, None]
             * p["out_proj"].astype(np.float32))
    return dict(
        xT=np.ascontiguousarray(x_seq.T).astype(ml_dtypes.bfloat16),
        w_in=np.ascontiguousarray(p["in_proj"]).astype(ml_dtypes.bfloat16),
        w_out=np.ascontiguousarray(w_out).astype(ml_dtypes.bfloat16),
        convw=convw,
        convb=convb,
        cwdiag=cwdiag.astype(ml_dtypes.bfloat16),
        dtb=p["dt_bias"].astype(np.float32).reshape(16, 1),
        nae=(-np.exp(p["A_log"].astype(np.float32))).reshape(16, 1),
        dcol=dcol,
        **consts,
    )


def kernel(**inputs):
    from concourse.bass_utils import run_bass_kernel_spmd

    nc = _get_program()
    x = np.asarray(inputs["x"], np.float32)
    mask = np.asarray(inputs["padding_mask"])

    def params(pre):
        names = ["in_proj", "conv_w", "conv_b", "dt_bias", "A_log", "D", "norm_w", "out_proj"]
        return {n: np.asarray(inputs[pre + n]) for n in names}

    pf, pb = params("fw_"), params("bw_")
    in_maps = []
    for b in range(B_SZ):
        in_maps.append(_core_inputs(x[b], pf))
    for b in range(B_SZ):
        in_maps.append(_core_inputs(x[b][::-1], pb))

    res = run_bass_kernel_spmd(nc, in_maps, core_ids=list(range(8)))
    out = np.empty((B_SZ, L, D_MODEL), np.float32)
    for b in range(B_SZ):
        yf = res.results[b]["yT"].T
        yb = res.results[B_SZ + b]["yT"].T[::-1]
        out[b] = yf + yb
    out[mask] = 0.0
    return out


# revision 30
# speedup vs baseline: 1.2371x; 1.2371x over previous
"""Bidirectional Mamba2 layer on 8 NeuronCores.

Sharding: 8 cores = 4 batch elements x 2 directions (fw/bw). Each core runs
one full Mamba2 layer pass on one sequence; the host flips the bw sequences,
adds fw+bw results, and applies the padding mask.

Per-core kernel (L=2048, chunked SSD scan with T=128), v2:
  1. in_proj as channel-major matmuls (bf16): silu(z), pre-conv xBC, dt.
  2. depthwise conv width-4 on Vector (overlapped under in_proj PE work),
     D-residual yD = D*x precomputed globally.
  3. DRAM roundtrip with DMA-transpose for time-major x, B per chunk.
  4. chunked scan with batched broadcast ops: per chunk one utmp build
     (V+G split), one seg-sum matmul group, batched exp, one masked-G
     multiply for M, batched state matmul, C consumed channel-major with
     inter-chunk decay applied post-PSUM via a DMA-broadcast a_bc tile.
  5. gating + RMSNorm interleaved per chunk (norm_w folded into out_proj
     weights on the host), out_proj at the tail.
"""

import numpy as np

D_MODEL = 512
D_STATE = 128
NH = 16
HD = 64
D_INNER = 1024
D_XBC = 1280
D_IN = 2320
L = 2048
T = 128
NCH = L // T
B_SZ = 4
EPS = 1e-5

_CACHE = {}


def _patch_drain(tile, mybir, ScopedClock):
    # workaround: this walrus build rejects >2 sem waits per instruction;
    # spread the TileContext exit-drain waits across nop instructions.
    def _drain_and_barrier(self, tick_clock, wait_clock):
        nc_ = self.nc
        probe = nc_.sync.nop()
        wait_clock.add_sem_waits(
            probe.ins, ScopedClock({None: tick_clock.global_clock})
        )
        waits = list(probe.ins.sync_info.on_wait or [])
        if probe.ins.sync_info is not None:
            probe.ins.sync_info.on_wait = waits[:1]
            rest = waits[1:]
        else:
            rest = []
        for w in rest:
            n = nc_.sync.nop()
            if n.ins.sync_info is None:
                n.ins.sync_info = mybir.SyncInfo(on_wait=[w], on_update=[])
            else:
                n.ins.sync_info.on_wait = [w]
        nc_.sync.drain()
        nc_.all_engine_barrier()
        assert self.sems is not None
        popped = nc_._tile_sem_poison_stack.pop()
        assert popped is self._sem_poison
        nc_.clear_and_free_semaphores(list(self.sems.allocated().values()))
        nc_.all_engine_barrier()

    tile.TileContext._drain_and_barrier = _drain_and_barrier


def _build_program():
    import concourse.bass as bass
    import concourse.mybir as mybir
    import concourse.tile as tile
    from concourse.vector_clock import ScopedClock

    _patch_drain(tile, mybir, ScopedClock)

    f32 = mybir.dt.float32
    bf16 = mybir.dt.bfloat16
    AF = mybir.ActivationFunctionType
    OP = mybir.AluOpType

    nc = bass.Bass("TRN2", target_bir_lowering=False, debug=False)

    # ---------------- DRAM I/O ----------------
    xT_d = nc.dram_tensor("xT", [D_MODEL, L], bf16, kind="ExternalInput")
    w_in_d = nc.dram_tensor("w_in", [D_MODEL, D_IN], bf16, kind="ExternalInput")
    w_out_d = nc.dram_tensor("w_out", [D_INNER, D_MODEL], bf16, kind="ExternalInput")
    cwdiag_d = nc.dram_tensor("cwdiag", [128, 32, 128], bf16, kind="ExternalInput")
    idnb_d = nc.dram_tensor("idnb", [128, 128], bf16, kind="ExternalInput")
    convw_d = nc.dram_tensor("convw", [128, 10, 4], f32, kind="ExternalInput")
    convb_d = nc.dram_tensor("convb", [128, 10], f32, kind="ExternalInput")
    dtb_d = nc.dram_tensor("dtb", [16, 1], f32, kind="ExternalInput")
    nae_d = nc.dram_tensor("nae", [16, 1], f32, kind="ExternalInput")  # -exp(A_log)
    dcol_d = nc.dram_tensor("dcol", [128, 8], f32, kind="ExternalInput")  # D per pair-tile
    alow_d = nc.dram_tensor("alow", [128, 128], bf16, kind="ExternalInput")
    uinc_d = nc.dram_tensor("uinc", [128, 128], bf16, kind="ExternalInput")
    idnf_d = nc.dram_tensor("idnf", [128, 128], f32, kind="ExternalInput")
    ones_d = nc.dram_tensor("ones", [128, 1], bf16, kind="ExternalInput")
    onesrf_d = nc.dram_tensor("onesrf", [1, 128], f32, kind="ExternalInput")
    onesrb_d = nc.dram_tensor("onesrb", [1, 128], bf16, kind="ExternalInput")
    yT_d = nc.dram_tensor("yT", [D_MODEL, L], f32, kind="ExternalOutput")

    with tile.TileContext(nc) as tc:
        with (
            tc.tile_pool(name="const", bufs=1) as cpool,
            tc.tile_pool(name="dram", bufs=1, space="DRAM") as dpool,
            tc.tile_pool(name="mid", bufs=1) as mid,
        ):
            # ---------------- constants ----------------
            ALOW = cpool.tile([128, 128], bf16, tag="alow")
            nc.sync.dma_start(ALOW[:], alow_d.ap())
            UINC = cpool.tile([128, 128], bf16, tag="uinc")
            nc.sync.dma_start(UINC[:], uinc_d.ap())
            IDNB = cpool.tile([128, 128], bf16, tag="idnb")
            nc.sync.dma_start(IDNB[:], idnb_d.ap())
            CWDIAG = cpool.tile([128, 32, 128], bf16, tag="cwdiag")
            nc.sync.dma_start(CWDIAG[:], cwdiag_d.ap())
            IDNF = cpool.tile([128, 128], f32, tag="idnf")
            nc.sync.dma_start(IDNF[:], idnf_d.ap())
            ONEC = cpool.tile([128, 1], bf16, tag="ones")
            nc.sync.dma_start(ONEC[:], ones_d.ap())
            ONESRF = cpool.tile([1, 128], f32, tag="onesrf")
            nc.sync.dma_start(ONESRF[:], onesrf_d.ap())
            ONESRB = cpool.tile([1, 128], bf16, tag="onesrb")
            nc.sync.dma_start(ONESRB[:], onesrb_d.ap())
            CONVW = cpool.tile([128, 10, 4], f32, tag="convw")
            nc.sync.dma_start(CONVW[:], convw_d.ap())
            CONVB = cpool.tile([128, 10], f32, tag="convb")
            nc.sync.dma_start(CONVB[:], convb_d.ap())
            DTB = cpool.tile([16, 1], f32, tag="dtb")
            nc.sync.dma_start(DTB[:], dtb_d.ap())
            NAE = cpool.tile([16, 1], f32, tag="nae")
            nc.sync.dma_start(NAE[:], nae_d.ap())
            DCOL = cpool.tile([128, 8], f32, tag="dcol")
            nc.sync.dma_start(DCOL[:], dcol_d.ap())
            EPSC = cpool.tile([128, 1], f32, tag="epsc")
            nc.vector.memset(EPSC[:], EPS)

            # ---------------- persistent mid tensors ----------------
            dtldT = mid.tile([128, NCH, 96], f32, tag="dtldT")  # time-major per chunk
            atot = mid.tile([16, 16], f32, tag="atot")          # [head, chunk]
            atotT = mid.tile([16, 16], f32, tag="atotT")
            atotF = mid.tile([1, 256], f32, tag="atotF")
            s_sb = [mid.tile([128, NH, HD], bf16, tag=f"s_sb{i}", name=f"s_sb{i}")
                    for i in range(2)]
            atb_all = mid.tile([128, NCH, 16], f32, tag="atb_all")    # atot bcast
            sz = mid.tile([128, 8, L], bf16, tag="sz")                # silu(z)
            rstd_cols = mid.tile([128, 16], f32, tag="rstd_cols")

            rt_dram = dpool.tile([2 * D_INNER + D_STATE, L], bf16)  # xdt,xdw,B
            aT_dram = dpool.tile([NCH, 16, 128], bf16)          # prefix decay aT
            suf_dram = dpool.tile([NCH, 16, 128], bf16)         # suffix decay
            dt_dram = dpool.tile([16, L], bf16)                 # dt rows
            yD_dram = dpool.tile([D_INNER, L], bf16)            # D*x residual

            xbc_B = mid.tile([128, L], bf16, tag="xbc_B")
            xbc_C = mid.tile([128, L], bf16, tag="xbc_C")

            # ============ PHASE 1: in_proj + conv + decay prep ============
            with tc.tile_pool(name="p_ph1", bufs=1) as p1:
                dtld = p1.tile([96, L], f32, tag="dtld")  # dt 0:16, logdA 64:80
                xbc_pre = [p1.tile([128, L + 3], bf16, tag=f"xbc_pre{t}",
                                   name=f"xbc_pre{t}") for t in range(10)]
                with (
                    tc.tile_pool(name="pA", bufs=1) as pA,
                    tc.tile_pool(name="ps1", bufs=4, space="PSUM") as ps1,
                    tc.tile_pool(name="pss", bufs=2, space="PSUM") as pss,
                    tc.tile_pool(name="pC", bufs=2) as pC,
                    tc.tile_pool(name="pC1", bufs=1) as pC1,
                    tc.tile_pool(name="pD", bufs=2) as pDp,
                    tc.tile_pool(name="pDT", bufs=1) as pDT,
                ):
                    xTr = xT_d.ap().rearrange("(ko p) t -> p ko t", p=128)
                    wir = w_in_d.ap().rearrange("(ko p) m -> p ko m", p=128)
                    xTs = pA.tile([128, 4, L], bf16, tag="xTs")
                    wis = pA.tile([128, 4, D_IN], bf16, tag="wis")
                    for k in range(4):
                        nc.sync.dma_start(xTs[:, k, :], xTr[:, k, :])
                        nc.sync.dma_start(wis[:, k, :], wir[:, k, :])

                    for t in range(10):
                        nc.vector.memset(xbc_pre[t][:, 0:3], 0.0)

                    def do_conv_x(t):
                        # x tiles: vector conv chain
                        acc = pC1.tile([128, L], bf16, tag="conv_acc")
                        nc.vector.tensor_scalar_mul(
                            acc[:], xbc_pre[t][:, 0:L], CONVW[:, t, 0:1])
                        for k in range(1, 4):
                            nc.vector.scalar_tensor_tensor(
                                acc[:], xbc_pre[t][:, k:k + L],
                                CONVW[:, t, k:k + 1], acc[:],
                                op0=OP.mult, op1=OP.add,
                            )
                        dst = pC.tile([128, L], bf16, tag="conv_x")
                        nc.scalar.activation(dst[:], acc[:], AF.Silu,
                                             bias=CONVB[:, t:t + 1])
                        # xdt = x*dt ; xdw = xdt*suffix  (channel-major, 2x TT)
                        dtc = pC1.tile([128, L], bf16, tag="dt_cm")
                        for hh in range(2):
                            nc.sync.dma_start(
                                dtc[hh * 64:(hh + 1) * 64, :],
                                dt_dram[2 * t + hh][None, :].to_broadcast([64, L]))
                        sfc = pC1.tile([128, NCH, 128], bf16, tag="suf_cm")
                        for hh in range(2):
                            nc.sync.dma_start(
                                sfc[hh * 64:(hh + 1) * 64, :, :],
                                suf_dram[:, 2 * t + hh, :][None, :, :]
                                .to_broadcast([64, NCH, 128]))
                        xdt = pC.tile([128, L], bf16, tag="xdt_cm")
                        nc.vector.tensor_tensor(xdt[:], dst[:], dtc[:], op=OP.mult)
                        xdw = pC1.tile([128, L], bf16, tag="xdw_cm")
                        nc.vector.tensor_tensor(
                            xdw[:], xdt[:],
                            sfc[:].rearrange("p c i -> p (c i)"), op=OP.mult)
                        nc.sync.dma_start(rt_dram[t * 128:(t + 1) * 128, :], xdt[:])
                        nc.sync.dma_start(
                            rt_dram[1024 + t * 128:1024 + (t + 1) * 128, :], xdw[:])
                        # D residual via DRAM
                        yDt = pC1.tile([128, L], bf16, tag="conv_yd")
                        nc.vector.tensor_scalar_mul(
                            yDt[:], dst[:], DCOL[:, t:t + 1])
                        nc.sync.dma_start(yD_dram[t * 128:(t + 1) * 128, :], yDt[:])

                    def do_conv_bc(t):
                        # B/C tiles: vector chain
                        acc = pC1.tile([128, L], bf16, tag="conv_acc")
                        nc.vector.tensor_scalar_mul(
                            acc[:], xbc_pre[t][:, 0:L], CONVW[:, t, 0:1])
                        for k in range(1, 4):
                            nc.vector.scalar_tensor_tensor(
                                acc[:], xbc_pre[t][:, k:k + L],
                                CONVW[:, t, k:k + 1], acc[:],
                                op0=OP.mult, op1=OP.add,
                            )
                        dst = xbc_B if t == 8 else xbc_C
                        nc.scalar.activation(dst[:], acc[:], AF.Silu,
                                             bias=CONVB[:, t:t + 1])
                        if t == 8:
                            nc.sync.dma_start(rt_dram[2048:2176, :], dst[:])

                    for m in [18] + list(range(8, 18)) + list(range(0, 8)):
                        mp = 128 if m < 18 else 16
                        if m < 18:
                            # stationary (m,k) reused across the 4 tb blocks
                            pstb = [ps1.tile([128, 512], f32, tag="ps_inproj",
                                             name=f"ip{m}_{tb}") for tb in range(4)]
                            for k in range(4):
                                for tb in range(4):
                                    nc.tensor.matmul(
                                        pstb[tb][:],
                                        wis[:, k, m * 128: m * 128 + mp],
                                        xTs[:, k, tb * 512:(tb + 1) * 512],
                                        start=(k == 0), stop=(k == 3))
                            for tb in range(4):
                                tsl = slice(tb * 512, (tb + 1) * 512)
                                if m < 8:
                                    nc.scalar.activation(sz[:, m, tsl], pstb[tb][:],
                                                         AF.Silu)
                                else:
                                    t = m - 8
                                    nc.scalar.copy(
                                        xbc_pre[t][:, 3 + tb * 512: 3 + (tb + 1) * 512],
                                        pstb[tb][:])
                        else:
                            for tb in range(4):
                                tsl = slice(tb * 512, (tb + 1) * 512)
                                ps = ps1.tile([128, 512], f32, tag="ps_inproj")
                                for k in range(4):
                                    nc.tensor.matmul(
                                        ps[:mp, :],
                                        wis[:, k, m * 128: m * 128 + mp],
                                        xTs[:, k, tsl],
                                        start=(k == 0), stop=(k == 3))
                                nc.scalar.copy(dtld[32:48, tsl], ps[:16, :])
                        if m == 18:
                            # dt = softplus(pre) = ln(1 + exp(pre + dtb))
                            nc.scalar.activation(dtld[32:48, :], dtld[32:48, :], AF.Exp,
                                                 bias=DTB[:, 0:1])
                            nc.scalar.activation(dtld[0:16, :], dtld[32:48, :], AF.Ln,
                                                 bias=1.0)
                            # logdA = -exp(A_log) * dt   (f32)
                            nc.vector.tensor_scalar_mul(
                                dtld[64:80, :], dtld[0:16, :], NAE[:, 0:1])
                            # dt rows to DRAM for channel-major broadcast
                            dtbf = pDT.tile([16, L], bf16, tag="dtbf")
                            nc.vector.tensor_copy(dtbf[:], dtld[0:16, :])
                            nc.sync.dma_start(dt_dram[:], dtbf[:])

                            # Atot per chunk = exp(chunk-sums of logdA)
                            red = pss.tile([128, 32], f32, tag="small", name="red")
                            nc.vector.tensor_reduce(
                                red[0:16, 0:16],
                                dtld[64:80, :].rearrange("p (c t) -> p c t", c=NCH),
                                op=OP.add, axis=mybir.AxisListType.X,
                            )
                            nc.scalar.activation(atot[:], red[0:16, 0:16], AF.Exp)
                            atT_ps = pss.tile([128, 32], f32, tag="small", name="atT_ps")
                            nc.tensor.transpose(
                                atT_ps[0:16, 0:16], atot[:], IDNF[0:16, 0:16])
                            nc.vector.tensor_copy(atotT[:], atT_ps[0:16, 0:16])
                            nc.sync.dma_start(
                                atotF[:].rearrange("p (c h) -> p c h", c=16), atotT[:])

                            # time-major dt/logdA per chunk via PE transpose
                            for c in range(NCH):
                                trp = pss.tile([128, 128], f32, tag="small2", name="trp")
                                nc.tensor.transpose(
                                    trp[:, 0:96], dtld[:, c * T:(c + 1) * T],
                                    IDNF[0:96, 0:96])
                                nc.vector.tensor_copy(dtldT[:, c, :], trp[:, 0:96])

                            # ---- decay prep (all chunks up front) ----
                            for c in range(NCH):
                                ld_bf = pDp.tile([128, 16], bf16, tag="ld_bf")
                                nc.vector.tensor_copy(ld_bf[:], dtldT[:, c, 64:80])
                                # suffix decay sufT[h,t]=exp(sum_{k>t} logdA)
                                sfT_ps = pss.tile([128, 128], f32, tag="small2",
                                                  name="sfT_ps")
                                nc.tensor.matmul(sfT_ps[0:16, :], ld_bf[:], ALOW[:],
                                                 start=True, stop=True)
                                sfT_sb = pDp.tile([16, 128], bf16, tag="sfT_sb")
                                nc.scalar.activation(sfT_sb[:], sfT_ps[0:16, :], AF.Exp)
                                nc.sync.dma_start(suf_dram[c], sfT_sb[:])
                                # prefix decay aT[h, t] = exp(inclusive cumsum)
                                csT_ps = pss.tile([128, 128], f32, tag="small2",
                                                  name="csT_ps")
                                nc.tensor.matmul(csT_ps[0:16, :], ld_bf[:], UINC[:],
                                                 start=True, stop=True)
                                aT_sb = pDp.tile([16, 128], bf16, tag="aT_sb")
                                nc.scalar.activation(aT_sb[:], csT_ps[0:16, :], AF.Exp)
                                nc.sync.dma_start(aT_dram[c], aT_sb[:])
                                if c > 0:
                                    at_ps = pss.tile([128, 32], f32, tag="small",
                                                     name="at_ps")
                                    nc.tensor.matmul(
                                        at_ps[:, 0:16], ONESRF[:],
                                        atotF[0:1, c * 16:(c + 1) * 16],
                                        start=True, stop=True)
                                    nc.vector.tensor_copy(atb_all[:, c, :],
                                                          at_ps[:, 0:16])

                    # conv interleaved (tiles ready in production order)
                    for t in range(8):
                        do_conv_x(t)
                    do_conv_bc(8)
                    do_conv_bc(9)

            # ============ PHASE 2: chunked scan + gating ============
            with tc.tile_pool(name="p_gn", bufs=1) as pgn:
              gn = pgn.tile([128, 8, L], bf16, tag="gn")
              with (
                tc.tile_pool(name="pS", bufs=2) as pS,
                tc.tile_pool(name="pXT", bufs=3) as pXT,
                tc.tile_pool(name="pAB", bufs=2) as pAB,
                tc.tile_pool(name="psE", bufs=1, space="PSUM") as psE,
                tc.tile_pool(name="psY1", bufs=1, space="PSUM") as psY1,
                tc.tile_pool(name="psY2", bufs=1, space="PSUM") as psY2,
                tc.tile_pool(name="psS", bufs=1, space="PSUM") as psS,
                tc.tile_pool(name="psR", bufs=1, space="PSUM") as psR,
              ):
                gn_prev = [None]

                def emit_gn(cp, g_p, rb_p):
                    # gn = g * rstd (norm_w folded into W_out on host)
                    nc.vector.tensor_tensor(
                        gn[:, :, cp * T:(cp + 1) * T], g_p[:],
                        rb_p[:, None, :].to_broadcast([128, 8, T]), op=OP.mult)

                for c in range(NCH):
                    csl = slice(c * T, (c + 1) * T)

                    # time-major xdt, xdw, B for this chunk
                    xbt = pXT.tile([128, 2 * D_INNER + D_STATE], bf16, tag="xbt")
                    nc.sync.dma_start_transpose(xbt[:, 0:1024], rt_dram[0:1024, csl])
                    nc.sync.dma_start_transpose(xbt[:, 1024:2048],
                                                rt_dram[1024:2048, csl])
                    nc.sync.dma_start_transpose(xbt[:, 2048:2176],
                                                rt_dram[2048:2176, csl])
                    yD_c = pXT.tile([128, 8, T], bf16, tag="yD_c")
                    nc.sync.dma_start(
                        yD_c[:],
                        yD_dram[:].rearrange("(t8 p) l -> p t8 l", p=128)[:, :, csl])

                    # a_bc[p, t8, i] = aT[2*t8 + p//64, i]
                    if c > 0:
                        a_bc = pAB.tile([128, 8, 128], bf16, tag="a_bc")
                        for hh in range(2):
                            nc.sync.dma_start(
                                a_bc[hh * 64:(hh + 1) * 64, :, :],
                                aT_dram[c, hh::2, :][None, :, :]
                                .to_broadcast([64, 8, 128]))

                    # deferred gn of previous chunk
                    if gn_prev[0] is not None:
                        emit_gn(*gn_prev[0])
                        gn_prev[0] = None

                    # utmp[k, h, i] = uinc[k, i] * logdA[k, h]  (V half, G half)
                    utmp = pS.tile([128, NH, 128], bf16, tag="utmp")
                    nc.vector.tensor_tensor(
                        utmp[:, 0:8, :],
                        UINC[:, None, :].to_broadcast([128, 8, 128]),
                        dtldT[:, c, 64:72, None].to_broadcast([128, 8, 128]),
                        op=OP.mult)
                    nc.gpsimd.tensor_tensor(
                        utmp[:, 8:16, :],
                        UINC[:, None, :].to_broadcast([128, 8, 128]),
                        dtldT[:, c, 72:80, None].to_broadcast([128, 8, 128]),
                        op=OP.mult)

                    # Gt = B^T C (shared across heads), masked to lower-incl
                    gt_ps = psE.tile([128, 512], f32, tag="psE", name="gt_ps")
                    nc.tensor.matmul(gt_ps[:, 0:128], xbc_B[:, csl], xbc_C[:, csl],
                                     start=True, stop=True)
                    gtm = pS.tile([128, 128], bf16, tag="gtm")
                    nc.vector.tensor_tensor(gtm[:], gt_ps[:, 0:128], UINC[:],
                                            op=OP.mult)

                    # segment sums + exp -> E, then M = gtm * E
                    e_all = pS.tile([128, NH, 128], bf16, tag="e_all")
                    for q in range(4):
                        e_ps = psE.tile([128, 512], f32, tag="psE", name="e_ps")
                        nc.tensor.matmul(e_ps[:], ALOW[:],
                                         utmp[:, 4 * q:4 * (q + 1), :],
                                         start=True, stop=True)
                        nc.scalar.activation(e_all[:, 4 * q:4 * (q + 1), :],
                                             e_ps[:], AF.Exp)
                    m_all = pS.tile([128, NH, 128], bf16, tag="m_all")
                    nc.vector.tensor_tensor(
                        m_all[:], e_all[:],
                        gtm[:, None, :].to_broadcast([128, NH, 128]), op=OP.mult)

                    # Y accumulation
                    y1_ps = psY1.tile([128, 8, T], f32, tag="y1_ps")
                    for h in range(NH):
                        ph, fh = (h % 2) * 64, h // 2
                        nc.tensor.matmul(y1_ps[ph:ph + 64, fh, :],
                                         xbt[:, h * HD:(h + 1) * HD],
                                         m_all[:, h, :],
                                         start=True, stop=True)
                    if c > 0:
                        y2_ps = psY2.tile([128, 8, T], f32, tag="y2_ps")
                        for h in range(NH):
                            ph, fh = (h % 2) * 64, h // 2
                            nc.tensor.matmul(y2_ps[ph:ph + 64, fh, :],
                                             s_sb[(c + 1) % 2][:, h, :],
                                             xbc_C[:, csl],
                                             start=True, stop=True)

                    # state: S = B^T @ xdw, then + S_prev*atot
                    s_ps = psS.tile([128, NH, HD], f32, tag="s_ps", name="s_ps")
                    for half in range(2):
                        hsl = slice(8 * half, 8 * (half + 1))
                        nc.tensor.matmul(
                            s_ps[:, hsl, :].rearrange("p h d -> p (h d)"),
                            xbt[:, 2048:2176],
                            xbt[:, 1024 + 512 * half:1024 + 512 * (half + 1)],
                            start=True, stop=True)
                    if c == 0:
                        nc.vector.tensor_copy(s_sb[0][:], s_ps[:])
                    else:
                        s_scaled = pS.tile([128, NH, HD], bf16, tag="s_scaled")
                        nc.gpsimd.tensor_tensor(
                            s_scaled[:], s_sb[(c + 1) % 2][:],
                            atb_all[:, c, :, None].to_broadcast([128, NH, HD]),
                            op=OP.mult)
                        nc.vector.tensor_tensor(
                            s_sb[c % 2][:], s_scaled[:], s_ps[:], op=OP.add)

                    # evac: y = y1 + a_bc*y2 + yD, then gate g = y*sz
                    g_sb = pS.tile([128, 8, T], bf16, tag="g_sb")
                    if c > 0:
                        y2s = pS.tile([128, 8, T], bf16, tag="y2s")
                        nc.vector.tensor_tensor(y2s[:], y2_ps[:], a_bc[:],
                                                op=OP.mult)
                        tmp = pS.tile([128, 8, T], bf16, tag="tmp")
                        nc.vector.tensor_tensor(tmp[:], y1_ps[:], y2s[:], op=OP.add)
                        yf = pS.tile([128, 8, T], bf16, tag="yf")
                        nc.vector.tensor_tensor(yf[:], tmp[:], yD_c[:], op=OP.add)
                    else:
                        yf = pS.tile([128, 8, T], bf16, tag="yf")
                        nc.vector.tensor_tensor(yf[:], y1_ps[:], yD_c[:], op=OP.add)
                    if c % 2 == 0:
                        nc.gpsimd.tensor_tensor(g_sb[:], yf[:], sz[:, :, csl],
                                                op=OP.mult)
                    else:
                        nc.vector.tensor_tensor(g_sb[:], yf[:], sz[:, :, csl],
                                                op=OP.mult)

                    # RMS stats: rstd per time (PE partition-reduce)
                    g2 = pS.tile([128, 8, T], bf16, tag="g2")
                    nc.scalar.square(g2[:], g_sb[:])
                    ss_ps = psR.tile([128, 512], f32, tag="r_ps", name="ss_ps")
                    for t8 in range(8):
                        nc.tensor.matmul(ss_ps[:, 0:1], g2[:, t8, :], ONEC[:],
                                         start=(t8 == 0), stop=(t8 == 7))
                    lnv = pS.tile([128, 1], f32, tag="lnv")
                    nc.scalar.activation(lnv[:], ss_ps[:, 0:1], AF.Ln,
                                         bias=EPSC[:, 0:1], scale=1.0 / D_INNER)
                    nc.scalar.activation(rstd_cols[:, c:c + 1], lnv[:],
                                         AF.Exp, scale=-0.5)
                    # broadcast rstd over partitions: transpose + 1-row matmul
                    rs_ps = psR.tile([128, 512], f32, tag="r_ps", name="rs_ps")
                    nc.tensor.transpose(rs_ps[0:1, 0:128],
                                        rstd_cols[:, c:c + 1], IDNF[:])
                    rsT = pS.tile([1, 128], bf16, tag="rsT")
                    nc.vector.tensor_copy(rsT[:], rs_ps[0:1, 0:128])
                    rb_ps = psR.tile([128, 512], f32, tag="r_ps", name="rb_ps")
                    nc.tensor.matmul(rb_ps[:, 0:128], ONESRB[:], rsT[:],
                                     start=True, stop=True)
                    rb_sb = pS.tile([128, 128], bf16, tag="rb_sb")
                    nc.vector.tensor_copy(rb_sb[:], rb_ps[:, 0:128])
                    gn_prev[0] = (c, g_sb, rb_sb)

                # final chunk's gn
                if gn_prev[0] is not None:
                    emit_gn(*gn_prev[0])
                    gn_prev[0] = None

              # ============ PHASE 3: out_proj ============
              with (
                tc.tile_pool(name="pO", bufs=1) as pO,
                tc.tile_pool(name="psO", bufs=4, space="PSUM") as psO,
              ):
                wo = pO.tile([128, 8, D_MODEL], bf16, tag="wo")
                nc.sync.dma_start(
                    wo[:], w_out_d.ap().rearrange("(k p) m -> p k m", p=128))
                yT_sb = pO.tile([128, 4, L], f32, tag="yT_sb")
                for m in range(4):
                    pstb = [psO.tile([128, 512], f32, tag="ps_out",
                                     name=f"op{m}_{tb}") for tb in range(4)]
                    for k in range(8):
                        for tb in range(4):
                            nc.tensor.matmul(
                                pstb[tb][:], wo[:, k, m * 128:(m + 1) * 128],
                                gn[:, k, tb * 512:(tb + 1) * 512],
                                start=(k == 0), stop=(k == 7))
                    for tb in range(4):
                        nc.scalar.copy(yT_sb[:, m, tb * 512:(tb + 1) * 512],
                                       pstb[tb][:])
                nc.sync.dma_start(
                    yT_d.ap().rearrange("(mo p) t -> p mo t", p=128), yT_sb[:])

    _fix_waits(nc, mybir)

    return nc


def _fix_waits(nc, mybir):
    """This walrus build supports one sem-wait slot per instruction; hoist
    excess waits onto preceding NoOps on the same engine."""
    nwn = [0]
    for bb in nc.main_func.blocks:
        newl = []
        changed = False
        for inst in bb.instructions:
            si = inst.sync_info
            waits = list(si.on_wait) if (si and si.on_wait) else []
            if len(waits) > 1:
                imm = [w for w in waits if w.wait_reg is None]
                reg = [w for w in waits if w.wait_reg is not None]
                keep = (reg + imm)[:1]
                spill = [w for w in waits if w not in keep]
                assert not any(w.wait_reg is not None for w in spill), inst.name
                for w in spill:
                    nwn[0] += 1
                    nop = mybir.InstNoOp(name=f"I-wsplit-{nwn[0]}", ins=[], outs=[])
                    nop.engine = inst.engine
                    nop.sync_info = mybir.SyncInfo(on_wait=[w], on_update=[])
                    nc.register_instruction(nop)
                    newl.append(nop)
                si.on_wait = keep
                changed = True
            newl.append(inst)
        if changed:
            bb.instructions = newl
    return nc


def _get_program():
    if "nc" not in _CACHE:
        _CACHE["nc"] = _build_program()
    return _CACHE["nc"]


def _host_consts():
    if "consts" in _CACHE:
        return _CACHE["consts"]
    import ml_dtypes
    k = np.arange(128)
    alow = (k[:, None] > k[None, :]).astype(np.float32)      # [k > j]
    uinc = (k[:, None] <= k[None, :]).astype(np.float32)     # [k <= i]
    idn = np.eye(128, dtype=np.float32)
    consts = dict(
        alow=alow.astype(ml_dtypes.bfloat16),
        uinc=uinc.astype(ml_dtypes.bfloat16),
        idnb=idn.astype(ml_dtypes.bfloat16),
        idnf=idn,
        ones=np.ones((128, 1), ml_dtypes.bfloat16),
        onesrf=np.ones((1, 128), np.float32),
        onesrb=np.ones((1, 128), ml_dtypes.bfloat16),
    )
    _CACHE["consts"] = consts
    return consts


def _core_inputs(x_seq, p):
    """x_seq: (L, D_MODEL) f32 (already flipped for bw); p: dict of params."""
    import ml_dtypes
    consts = _host_consts()
    dcol = p["D"].astype(np.float32).repeat(HD).reshape(8, 128).T.copy()
    convw = np.ascontiguousarray(
        p["conv_w"].astype(np.float32).reshape(4, 10, 128).transpose(2, 1, 0)
    )
    convb = np.ascontiguousarray(p["conv_b"].astype(np.float32).reshape(10, 128).T)
    # diag conv weights for the 8 x-tiles: cwdiag[p, t*4+k, m] = d_pm * w[k, t*128+p]
    cwdiag = np.zeros((128, 32, 128), np.float32)
    cw = p["conv_w"].astype(np.float32)  # [4, 1280]
    for t in range(8):
        for k in range(4):
            np.fill_diagonal(cwdiag[:, t * 4 + k, :], cw[k, t * 128:(t + 1) * 128])
    w_out = (p["norm_w"].astype(np.float32)[:, None]
             * p["out_proj"].astype(np.float32))
    return dict(
        xT=np.ascontiguousarray(x_seq.T).astype(ml_dtypes.bfloat16),
        w_in=np.ascontiguousarray(p["in_proj"]).astype(ml_dtypes.bfloat16),
        w_out=np.ascontiguousarray(w_out).astype(ml_dtypes.bfloat16),
        convw=convw,
        convb=convb,
        cwdiag=cwdiag.astype(ml_dtypes.bfloat16),
        dtb=p["dt_bias"].astype(np.float32).reshape(16, 1),
        nae=(-np.exp(p["A_log"].astype(np.float32))).reshape(16, 1),
        dcol=dcol,
        **consts,
    )


def kernel(**inputs):
    from concourse.bass_utils import run_bass_kernel_spmd

    nc = _get_program()
    x = np.asarray(inputs["x"], np.float32)
    mask = np.asarray(inputs["padding_mask"])

    def params(pre):
        names = ["in_proj", "conv_w", "conv_b", "dt_bias", "A_log", "D", "norm_w", "out_proj"]
        return {n: np.asarray(inputs[pre + n]) for n in names}

    pf, pb = params("fw_"), params("bw_")
    in_maps = []
    for b in range(B_SZ):
        in_maps.append(_core_inputs(x[b], pf))
    for b in range(B_SZ):
        in_maps.append(_core_inputs(x[b][::-1], pb))

    res = run_bass_kernel_spmd(nc, in_maps, core_ids=list(range(8)))
    out = np.empty((B_SZ, L, D_MODEL), np.float32)
    for b in range(B_SZ):
        yf = res.results[b]["yT"].T
        yb = res.results[B_SZ + b]["yT"].T[::-1]
        out[b] = yf + yb
    out[mask] = 0.0
    return out
